# revision 1
# baseline (speedup 1.0000x reference)
"""Fused LayerNorm + multi-head attention + out-projection for Trainium2.

Problem: x[2,2048,1024] -> LN -> QKV (16 heads, dh=64) -> softmax attention
-> out proj.  Sharded over 8 NeuronCores as batch(2) x head-groups(4)
(Megatron tensor parallel): each core handles one batch entry and 4 heads,
computing a partial out-projection; the host sums the 4 partials per batch.

Per-core dataflow (T=2048 tokens, D=1024, 4 local heads, dh=64):
  A/B) per 512-token chunk: LN in [tok, dim] layout (bn_stats/bn_aggr),
     PE-transpose to xnT [dim, tok]; v natural via lhsT=xnT chunks (+ones
     column); qT,kT = (xn @ wq/wk)^T via lhsT=w chunks.
  C) per head: S^T[j,i] = kT.T @ qT; exp(S/8) on ACT (no max-subtraction:
     |S/8| <= ~6); O^T[d,i] (+row of sums r) = (V|1).T @ expS^T; normalize
     by r via K=1 ones-matmul replicate + reciprocal, written into OT.
  D) out = OT.T @ w_out chunks.
All matmuls run in float32r (~1.5e-4 rel err), everything else fp32.
gamma is folded into w_qkv on the host; beta/b_out are zeros by spec
(b_out still added on the host).
"""
import numpy as np

import concourse.bacc as bacc
import concourse.mybir as mybir
import concourse.tile as tile
from concourse import bass_utils
from concourse.masks import make_identity

F32 = mybir.dt.float32
F32R = mybir.dt.float32r
AF = mybir.ActivationFunctionType
ALU = mybir.AluOpType

T = 2048          # tokens per core (one batch entry)
D = 1024          # model dim
HL = 4            # local heads per core
DH = 64           # head dim
CI = HL * DH      # local inner dim = 256
NT = T // 128     # 16 token tiles
NK = D // 128     # 8 dim chunks
LN_EPS = 1e-5
SCALE = DH ** -0.5

_NC_CACHE = {}


def _build(phases="full"):
    nc = bacc.Bacc("TRN2", target_bir_lowering=False, debug=False)

    x = nc.dram_tensor("x", [T, D], F32, kind="ExternalInput")
    wq = nc.dram_tensor("wq", [D, CI], F32, kind="ExternalInput")
    wk = nc.dram_tensor("wk", [D, CI], F32, kind="ExternalInput")
    wv = nc.dram_tensor("wv", [D, CI], F32, kind="ExternalInput")
    wo = nc.dram_tensor("wo", [CI, D], F32, kind="ExternalInput")
    out = nc.dram_tensor("out", [T, D], F32, kind="ExternalOutput")

    x_t = x.rearrange("(t p) d -> t p d", p=128)          # [16, 128, 1024]
    out_t = out.rearrange("(t p) d -> t p d", p=128)
    wq_t = wq.rearrange("(c p) n -> p c n", p=128)        # [128, 8, 256]
    wk_t = wk.rearrange("(c p) n -> p c n", p=128)
    wv_t = wv.rearrange("(c p) n -> p c n", p=128)
    wo_t = wo.rearrange("(c p) n -> p c n", p=128)        # [128, 2, 1024]

    _dma_engines = [nc.sync, nc.scalar]
    with tile.TileContext(nc) as tc:
        with (
            tc.tile_pool(name="persist", bufs=1) as persist,
            tc.tile_pool(name="g_ps", bufs=1, space="PSUM") as g_ps,
        ):
            # constants
            ident_f = persist.tile([128, 128], F32, name="ident_f")
            make_identity(nc, ident_f)
            ident = persist.tile([128, 128], F32R, name="ident")
            nc.vector.tensor_copy(out=ident, in_=ident_f)
            eps = persist.tile([128, 1], F32, name="eps")
            nc.vector.memset(eps, LN_EPS)
            ones1f = persist.tile([1, 128], F32, name="ones1f")
            nc.vector.memset(ones1f, 1.0)
            ones1 = persist.tile([1, 128], F32R, name="ones1")
            nc.vector.tensor_copy(out=ones1, in_=ones1f)

            # persistent activations / weights
            qkT = persist.tile([128, 4, T], F32R, name="qkT")        # 32KB/p
            vext = persist.tile([128, NT, HL, 65], F32R, name="vext")
            OT = persist.tile([128, 2, T], F32R, name="OT")          # 16KB/p
            wo_r = persist.tile([128, 2, D], F32R, name="wo_r")

            # ------------- Phase A/B: LN + transpose + QKV, ic-major -------------
            with (
                tc.tile_pool(name="ab_sb", bufs=3) as ab_sb,
                tc.tile_pool(name="ab_w", bufs=1) as ab_w,
                tc.tile_pool(name="ab_sm", bufs=8) as ab_sm,
            ):
                # prefetch the first token tiles before weight staging
                xts = {}
                for tt in range(4):
                    xt0 = ab_sb.tile([128, D], F32, tag="xt", name="xt", bufs=4)
                    _dma_engines[tt % len(_dma_engines)].dma_start(xt0, x_t[tt])
                    xts[tt] = xt0

                # round weights to f32r (staged through one fp32 buffer)
                wq_r = ab_w.tile([128, NK, CI], F32R, name="wq_r")
                wk_r = ab_w.tile([128, NK, CI], F32R, name="wk_r")
                wv_r = ab_w.tile([128, NK, CI], F32R, name="wv_r")
                for (src, dst) in ((wq_t, wq_r), (wk_t, wk_r), (wv_t, wv_r)):
                    stg = ab_sb.tile([128, NK, CI], F32, tag="wstg", name="stg", bufs=2)
                    nc.sync.dma_start(stg, src)
                    nc.vector.tensor_copy(out=dst, in_=stg)
                stg = ab_sb.tile([128, 2, D], F32, tag="wstg", name="stg", bufs=2)
                nc.sync.dma_start(stg, wo_t)
                nc.vector.tensor_copy(out=wo_r, in_=stg)

                # ones column of vext
                onev = ab_sm.tile([128, NT * HL], F32, tag="onev")
                nc.vector.memset(onev, 1.0)
                nc.vector.tensor_copy(
                    out=vext[:, :, :, 64],
                    in_=onev.rearrange("p (t h) -> p t h", t=NT),
                )

                for ic in range(4):
                    xnT_ic = ab_sb.tile([128, NK, 512], F32R, tag="xnTic",
                                        name="xnT_ic")
                    for tl in range(4):
                        tt = ic * 4 + tl
                        if tt in xts:
                            xt = xts.pop(tt)
                        else:
                            xt = ab_sb.tile([128, D], F32, tag="xt", name="xt",
                                            bufs=4)
                            _dma_engines[tt % len(_dma_engines)].dma_start(
                                xt, x_t[tt])
                        stats = ab_sm.tile([128, 2, 6], F32, tag="stats",
                                           name="stats")
                        xr = xt.rearrange("p (c f) -> p c f", f=512)
                        for c in range(2):
                            nc.vector.bn_stats(out=stats[:, c, :], in_=xr[:, c, :])
                        mv = ab_sm.tile([128, 2], F32, tag="mv", name="mv")
                        nc.vector.bn_aggr(out=mv, in_=stats)
                        rstd = ab_sm.tile([128, 1], F32, tag="rstd", name="rstd")
                        nc.scalar.activation(out=rstd, in_=mv[:, 1:2], func=AF.Sqrt,
                                             bias=eps, scale=1.0)
                        nc.vector.reciprocal(out=rstd, in_=rstd)
                        xn = ab_sb.tile([128, D], F32R, tag="xn", name="xn")
                        nc.vector.tensor_scalar(out=xn, in0=xt, scalar1=mv[:, 0:1],
                                                scalar2=rstd, op0=ALU.subtract,
                                                op1=ALU.mult)
                        for kc4 in range(NK // 4):
                            pt = g_ps.tile([128, 4, 128], F32R, tag="b1", name="pt", bufs=4)
                            for q in range(4):
                                nc.tensor.transpose(
                                    pt[:, q, :],
                                    xn[:, (kc4 * 4 + q) * 128:(kc4 * 4 + q + 1) * 128],
                                    ident)
                            nc.any.tensor_copy(
                                out=xnT_ic[:, kc4 * 4:kc4 * 4 + 4,
                                           tl * 128:(tl + 1) * 128], in_=pt)

                    # v natural for these 4 token tiles
                    for tl in range(4):
                        pv = g_ps.tile([128, CI], F32, tag="b1", name="pv", bufs=4)
                        for kc in range(NK):
                            nc.tensor.matmul(
                                pv,
                                lhsT=xnT_ic[:, kc, tl * 128:(tl + 1) * 128],
                                rhs=wv_r[:, kc, :],
                                start=(kc == 0), stop=(kc == NK - 1))
                        nc.any.tensor_copy(
                            out=vext[:, ic * 4 + tl, :, 0:64],
                            in_=pv.rearrange("p (h d) -> p h d", h=HL))

                    # qT/kT columns for this token chunk
                    sq = [g_ps.tile([128, 1024], F32, tag="s", name=f"sq{i}",
                                    bufs=2) for i in range(2)]
                    pq = [sq[i // 2][:, (i % 2) * 512:(i % 2 + 1) * 512]
                          for i in range(4)]
                    for kc in range(NK):
                        for pc in range(4):
                            w_src = wq_r if pc < 2 else wk_r
                            off = (pc % 2) * 128
                            nc.tensor.matmul(
                                pq[pc],
                                lhsT=w_src[:, kc, off:off + 128],
                                rhs=xnT_ic[:, kc, :],
                                start=(kc == 0), stop=(kc == NK - 1))
                    for pc in range(4):
                        nc.any.tensor_copy(
                            out=qkT[:, pc, ic * 512:(ic + 1) * 512], in_=pq[pc])

            if phases == "ab":
                with tc.tile_pool(name="anch", bufs=2) as anch:
                    a0 = anch.tile([128, D], F32, tag="a0", name="a0")
                    nc.vector.tensor_copy(out=a0, in_=qkT[:, 0, 0:1024].bitcast(F32))
                    nc.sync.dma_start(out_t[0], a0)
                    a1 = anch.tile([128, 780], F32, tag="a1", name="a1")
                    nc.vector.tensor_copy(out=a1, in_=vext[:, 0:3, :, :].bitcast(F32).rearrange("p a b c -> p (a b c)"))
                    nc.sync.dma_start(out_t[1][:, 0:780], a1)
                    a2 = anch.tile([128, D], F32, tag="a2", name="a2")
                    nc.vector.tensor_copy(out=a2, in_=qkT[:, 2, 0:1024].bitcast(F32))
                    nc.sync.dma_start(out_t[2], a2)

            # ---------------- Phase C: attention per head ----------------
            if phases != "ab":
              with (
                  tc.tile_pool(name="c_exp", bufs=3) as c_exp,
                  tc.tile_pool(name="c_sm", bufs=8) as c_sm,
              ):
                  r_all = c_sm.tile([1, HL * 4, 512], F32R, name="r_all", bufs=1)

                  def norm_one(ic, h):
                      # replicate r (K=1 ones matmul), reciprocal, scale O^T
                      po = (h % 2) * 64
                      qc = h // 2
                      ps_rr = g_ps.tile([128, 512], F32, tag="b1", name="ps_rr",
                                        bufs=4)
                      nc.tensor.matmul(ps_rr, lhsT=ones1,
                                       rhs=r_all[0:1, h * 4 + ic, :],
                                       start=True, stop=True)
                      rcp = c_sm.tile([128, 512], F32, tag="rcp", name="rcp",
                                      bufs=4)
                      nc.vector.reciprocal(out=rcp, in_=ps_rr)
                      sl = OT[po:po + 64, qc, ic * 512:(ic + 1) * 512]
                      nc.vector.tensor_tensor(out=sl, in0=sl,
                                              in1=rcp[po:po + 64, :],
                                              op=ALU.mult)

                  def d_one(tt):
                      # one out-projection token tile (shares C pools)
                      ot = c_exp.tile([128, 2, 512], F32, tag="e", name="ot",
                                      bufs=6)
                      pd = g_ps.tile([128, 1024], F32, tag="s", name="pd",
                                     bufs=2)
                      for ck in range(2):
                          for ncn in range(2):
                              nc.tensor.matmul(
                                  pd[:, ncn * 512:(ncn + 1) * 512],
                                  lhsT=OT[:, ck, tt * 128:(tt + 1) * 128],
                                  rhs=wo_r[:, ck, ncn * 512:(ncn + 1) * 512],
                                  start=(ck == 0), stop=(ck == 1))
                      nc.any.tensor_copy(out=ot,
                                         in_=pd.rearrange("p (a b) -> p a b", a=2))
                      _dma_engines[tt % len(_dma_engines)].dma_start(
                          out_t[tt], ot.rearrange("p a b -> p (a b)"))

                  # head pairs (2p, 2p+1) sit at partition offsets 0/64 of the
                  # same qkT chunk: alternating their S matmuls uses disjoint
                  # PE row groups, which overlap (~2x on K=64 matmuls).  Two
                  # passes over half the i-range each keep PSUM at 8 banks.
                  # Normalize + out-projection work for a finished i-half is
                  # queued and drip-fed between later jt iterations so it
                  # hides under the ACT-bound attention passes.
                  pending = []
                  for pr in range(2):              # head pair
                      qc = pr                      # chunk holding both heads' q
                      kcnk = 2 + pr                # chunk holding both heads' k
                      for half in range(2):        # i-range half (2 chunks)
                          ps_o = [g_ps.tile([65, 512], F32, tag="b1",
                                            name=f"o{i}", bufs=4)
                                  for i in range(4)]   # [head parity][i2]
                          for jt in range(NT):
                              for i2 in range(2):
                                  ic = half * 2 + i2
                                  # one psum tile: [head0 chunk | head1 chunk]
                                  ps_s = g_ps.tile([128, 1024], F32, tag="s",
                                                   name="ps_s", bufs=2)
                                  for hp in range(2):
                                      po = hp * 64
                                      nc.tensor.matmul(
                                          ps_s[:, hp * 512:(hp + 1) * 512],
                                          lhsT=qkT[po:po + 64, kcnk,
                                                   jt * 128:(jt + 1) * 128],
                                          rhs=qkT[po:po + 64, qc,
                                                  ic * 512:(ic + 1) * 512],
                                          start=True, stop=True)
                                  ex = c_exp.tile([128, 1024], F32R, tag="e",
                                                  name="ex", bufs=6)
                                  nc.scalar.activation(out=ex, in_=ps_s,
                                                       func=AF.Exp, scale=SCALE)
                                  for hp in range(2):
                                      nc.tensor.matmul(
                                          ps_o[hp * 2 + i2],
                                          lhsT=vext[:, jt, pr * 2 + hp, :],
                                          rhs=ex[:, hp * 512:(hp + 1) * 512],
                                          start=(jt == 0), stop=(jt == NT - 1),
                                          skip_group_check=True)
                          # stash r rows and unnormalized O^T (cheap DVE ops
                          # only, keeps the next pass unblocked on PE)
                          for i2 in range(2):
                              for hp in range(2):
                                  h = pr * 2 + hp
                                  ic = half * 2 + i2
                                  po = hp * 64
                                  nc.vector.tensor_copy(
                                      out=r_all[0:1, h * 4 + ic, :],
                                      in_=ps_o[hp * 2 + i2][64:65, :])
                                  nc.vector.tensor_copy(
                                      out=OT[po:po + 64, qc,
                                             ic * 512:(ic + 1) * 512],
                                      in_=ps_o[hp * 2 + i2][0:64, :])
                  # deferred normalization then out-projection, ic-major so
                  # phase D's token tiles unblock incrementally
                  for ic in range(4):
                      for h in range(HL):
                          norm_one(ic, h)
                  if phases == "full":
                      for tt in range(NT):
                          d_one(tt)

            if phases in ("abc", "abcn"):
                with tc.tile_pool(name="anch2", bufs=2) as anch2:
                    for ck in range(2):
                        b0 = anch2.tile([128, D], F32, tag="b0", name="b0")
                        nc.vector.tensor_copy(out=b0, in_=OT[:, ck, 0:1024].bitcast(F32))
                        nc.sync.dma_start(out_t[ck], b0)



    nc.compile()
    return nc


def kernel(x, gamma, beta, w_qkv, w_out, b_out):
    """Full inputs in, full output out.  Shards batch x head-groups over 8
    cores, runs the SPMD Bass kernel, and sums the partial projections."""
    if "nc" not in _NC_CACHE:
        _NC_CACHE["nc"] = _build()
    nc = _NC_CACHE["nc"]

    x = np.asarray(x, dtype=np.float32)
    gamma = np.asarray(gamma, dtype=np.float32)
    w_qkv = np.asarray(w_qkv, dtype=np.float32)
    w_out = np.asarray(w_out, dtype=np.float32)
    b_out = np.asarray(b_out, dtype=np.float32)

    wg = w_qkv * gamma[:, None]  # fold LN gamma into the QKV projection
    in_maps = []
    for core in range(8):
        b, g = core // 4, core % 4
        cs = slice(g * CI, (g + 1) * CI)
        in_maps.append({
            "x": np.ascontiguousarray(x[b]),
            "wq": np.ascontiguousarray(wg[:, 0 * 1024:1 * 1024][:, cs]),
            "wk": np.ascontiguousarray(wg[:, 1 * 1024:2 * 1024][:, cs]),
            "wv": np.ascontiguousarray(wg[:, 2 * 1024:3 * 1024][:, cs]),
            "wo": np.ascontiguousarray(w_out[cs, :]),
        })

    res = bass_utils.run_bass_kernel_spmd(nc, in_maps, core_ids=list(range(8)))
    parts = [r["out"] for r in res.results]
    full = np.stack([
        parts[0] + parts[1] + parts[2] + parts[3],
        parts[4] + parts[5] + parts[6] + parts[7],
    ]).astype(np.float32)
    return full + b_out



# revision 5
# speedup vs baseline: 1.1986x; 1.1986x over previous
"""Fused LayerNorm + multi-head attention + out-projection for Trainium2.

Problem: x[2,2048,1024] -> LN -> QKV (16 heads, dh=64) -> softmax attention
-> out proj.  Sharded over 8 NeuronCores as batch(2) x head-groups(4)
(Megatron tensor parallel): each core handles one batch entry and 4 heads,
computing a partial out-projection; the host sums the 4 partials per batch.

Per-core dataflow (T=2048 tokens, D=1024, 4 local heads, dh=64), bf16
matmul inputs everywhere (fp32 psum accumulation):
  A) per 128-token tile: LN via bn_stats/bn_aggr (fp32), xn cast to bf16,
     PE-transpose to xnT [d, tok].
  B) per 512-token chunk: v natural [tok, c] via lhsT=xnT; qT/kT [c, tok]
     via lhsT=w chunks.  All bf16.
  C) per (i-chunk 512, head-pair): for each j-tile: S^T[j,i] paired matmul
     (two heads on disjoint 64-row PE groups); exp((1/8)S) on ACT -> bf16;
     O[i, d|r] += ex_slice.T @ (V|1) for each 128-token i-sub -- M=128,
     N=66 bf16 matmuls accumulate over j in PSUM; softmax denominator r
     lands in column 64.  Normalize with per-partition scalar 1/r (DVE),
     PE-transpose O back to OT [c, i].
  D) per token tile: out = OT.T @ wo chunks, DMA out.
The exp stream on the scalar engine (~1 elem/cycle/lane) is the wall-clock
floor; all PE/DVE work is sized to hide under it.
gamma is folded into w_qkv on the host; beta/b_out are zeros by spec
(b_out still added on the host).
"""
import numpy as np

import concourse.bacc as bacc
import concourse.mybir as mybir
import concourse.tile as tile
from concourse import bass_utils
from concourse.masks import make_identity

F32 = mybir.dt.float32
BF16 = mybir.dt.bfloat16
AF = mybir.ActivationFunctionType
ALU = mybir.AluOpType

T = 2048          # tokens per core (one batch entry)
D = 1024          # model dim
HL = 4            # local heads per core
DH = 64           # head dim
CI = HL * DH      # local inner dim = 256
NT = T // 128     # 16 token tiles
NK = D // 128     # 8 dim chunks
LN_EPS = 1e-5
SCALE = DH ** -0.5

_NC_CACHE = {}


def _build():
    nc = bacc.Bacc("TRN2", target_bir_lowering=False, debug=False)

    x = nc.dram_tensor("x", [T, D], F32, kind="ExternalInput")
    wq = nc.dram_tensor("wq", [D, CI], BF16, kind="ExternalInput")
    wk = nc.dram_tensor("wk", [D, CI], BF16, kind="ExternalInput")
    wv = nc.dram_tensor("wv", [D, CI], BF16, kind="ExternalInput")
    wo = nc.dram_tensor("wo", [CI, D], BF16, kind="ExternalInput")
    out = nc.dram_tensor("out", [T, D], F32, kind="ExternalOutput")

    x_t = x.rearrange("(t p) d -> t p d", p=128)          # [16, 128, 1024]
    out_t = out.rearrange("(t p) d -> t p d", p=128)
    wq_t = wq.rearrange("(c p) n -> p c n", p=128)        # [128, 8, 256]
    wk_t = wk.rearrange("(c p) n -> p c n", p=128)
    wv_t = wv.rearrange("(c p) n -> p c n", p=128)
    wo_t = wo.rearrange("(c p) n -> p c n", p=128)        # [128, 2, 1024]

    with tile.TileContext(nc) as tc:
        with tc.tile_pool(name="persist", bufs=1) as persist:
            # constants
            ident_f = persist.tile([128, 128], F32, name="ident_f")
            make_identity(nc, ident_f)
            ident = persist.tile([128, 128], BF16, name="ident")
            nc.vector.tensor_copy(out=ident, in_=ident_f)
            eps = persist.tile([128, 1], F32, name="eps")
            nc.vector.memset(eps, LN_EPS)

            # persistent activations / weights (all bf16)
            qT = persist.tile([128, 2, T], BF16, name="qT")     # 8KB/p
            kT = persist.tile([128, 2, T], BF16, name="kT")     # 8KB/p
            vext = persist.tile([128, NT, HL, 66], BF16, name="vext")
            OT = persist.tile([128, 2, T], BF16, name="OT")     # 8KB/p
            wq_s = persist.tile([128, NK, CI], BF16, name="wq_s")
            wk_s = persist.tile([128, NK, CI], BF16, name="wk_s")
            wv_s = persist.tile([128, NK, CI], BF16, name="wv_s")
            wo_s = persist.tile([128, 2, D], BF16, name="wo_s")

            # ones column (64) of vext; column 65 is zero padding
            nc.vector.memset(vext[:, :, :, 64:65], 1.0)
            nc.vector.memset(vext[:, :, :, 65:66], 0.0)

            # ------------- Phase A/B: LN + transpose + QKV -------------
            with (
                tc.tile_pool(name="ab_sb", bufs=3) as ab_sb,
                tc.tile_pool(name="ab_sm", bufs=8) as ab_sm,
                tc.tile_pool(name="ab_ps", bufs=1, space="PSUM") as ab_ps,
            ):
                # prefetch first token tiles, then weights
                xts = {}
                for tt in range(4):
                    xt0 = ab_sb.tile([128, D], F32, tag="xt", name="xt", bufs=4)
                    (nc.sync if tt % 2 == 0 else nc.scalar).dma_start(
                        xt0, x_t[tt])
                    xts[tt] = xt0
                nc.sync.dma_start(wq_s, wq_t)
                nc.sync.dma_start(wk_s, wk_t)
                nc.sync.dma_start(wv_s, wv_t)
                nc.sync.dma_start(wo_s, wo_t)

                for ic in range(4):
                    xnT = ab_sb.tile([128, NK, 512], BF16, tag="xnT",
                                     name="xnT", bufs=2)
                    for tl in range(4):
                        tt = ic * 4 + tl
                        if tt in xts:
                            xt = xts.pop(tt)
                        else:
                            xt = ab_sb.tile([128, D], F32, tag="xt", name="xt",
                                            bufs=4)
                            (nc.sync if tt % 2 == 0 else nc.scalar).dma_start(
                                xt, x_t[tt])
                        stats = ab_sm.tile([128, 2, 6], F32, tag="stats",
                                           name="stats")
                        xr = xt.rearrange("p (c f) -> p c f", f=512)
                        for c in range(2):
                            nc.vector.bn_stats(out=stats[:, c, :], in_=xr[:, c, :])
                        mv = ab_sm.tile([128, 2], F32, tag="mv", name="mv")
                        nc.vector.bn_aggr(out=mv, in_=stats)
                        rstd = ab_sm.tile([128, 1], F32, tag="rstd", name="rstd")
                        nc.scalar.activation(out=rstd, in_=mv[:, 1:2], func=AF.Sqrt,
                                             bias=eps, scale=1.0)
                        nc.vector.reciprocal(out=rstd, in_=rstd)
                        xn = ab_sb.tile([128, D], BF16, tag="xn", name="xn")
                        nc.vector.tensor_scalar(out=xn, in0=xt, scalar1=mv[:, 0:1],
                                                scalar2=rstd, op0=ALU.subtract,
                                                op1=ALU.mult)
                        for kc4 in range(2):
                            pt = ab_ps.tile([128, 4, 128], BF16, tag="tp",
                                            name="pt", bufs=2)
                            for q in range(4):
                                kc = kc4 * 4 + q
                                nc.tensor.transpose(
                                    pt[:, q, :],
                                    xn[:, kc * 128:(kc + 1) * 128],
                                    ident)
                            nc.vector.tensor_copy(
                                out=xnT[:, kc4 * 4:kc4 * 4 + 4,
                                        tl * 128:(tl + 1) * 128], in_=pt)

                    # v natural for these 4 token tiles
                    for tl in range(4):
                        tt = ic * 4 + tl
                        pv = ab_ps.tile([128, CI], F32, tag="pv", name="pv",
                                        bufs=2)
                        for kc in range(NK):
                            nc.tensor.matmul(
                                pv,
                                lhsT=xnT[:, kc, tl * 128:(tl + 1) * 128],
                                rhs=wv_s[:, kc, :],
                                start=(kc == 0), stop=(kc == NK - 1))
                        nc.scalar.copy(
                            out=vext[:, tt, :, 0:64],
                            in_=pv.rearrange("p (h d) -> p h d", h=HL))

                    # qT/kT columns for this token chunk
                    pq = ab_ps.tile([128, 4, 512], F32, tag="pq", name="pq",
                                    bufs=1)
                    for kc in range(NK):
                        for pc in range(4):
                            w_src = wq_s if pc < 2 else wk_s
                            off = (pc % 2) * 128
                            nc.tensor.matmul(
                                pq[:, pc, :],
                                lhsT=w_src[:, kc, off:off + 128],
                                rhs=xnT[:, kc, :],
                                start=(kc == 0), stop=(kc == NK - 1))
                    isl = slice(ic * 512, (ic + 1) * 512)
                    nc.scalar.copy(out=qT[:, 0, isl], in_=pq[:, 0, :])
                    nc.scalar.copy(out=qT[:, 1, isl], in_=pq[:, 1, :])
                    nc.scalar.copy(out=kT[:, 0, isl], in_=pq[:, 2, :])
                    nc.scalar.copy(out=kT[:, 1, isl], in_=pq[:, 3, :])

            # ---------------- Phase C: attention ----------------
            with (
                tc.tile_pool(name="c_sb", bufs=1) as c_sb,
                tc.tile_pool(name="c_ps", bufs=1, space="PSUM") as c_ps,
            ):
                def d_one(tt):
                    # one out-projection token tile
                    pd = c_ps.tile([128, 1024], F32, tag="s", name="pd",
                                   bufs=2)
                    for ncn in range(2):
                        for ck in range(2):
                            nc.tensor.matmul(
                                pd[:, ncn * 512:(ncn + 1) * 512],
                                lhsT=OT[:, ck, tt * 128:(tt + 1) * 128],
                                rhs=wo_s[:, ck, ncn * 512:(ncn + 1) * 512],
                                start=(ck == 0), stop=(ck == 1))
                    ot_sb = c_sb.tile([128, 1024], F32, tag="ot", name="ot_sb",
                                      bufs=3)
                    nc.vector.tensor_copy(out=ot_sb, in_=pd)
                    nc.sync.dma_start(out_t[tt], ot_sb)

                def finish_ic(ic, o_nat):
                    # transpose normalized O [i, c] -> OT [c, i], then project
                    for isub in range(4):
                        tp2 = c_ps.tile([128, 2, 128], BF16, tag="tp2",
                                        name="tp2", bufs=2)
                        for ck in range(2):
                            nc.tensor.transpose(
                                tp2[:, ck, :],
                                o_nat[:, isub, 2 * ck:2 * ck + 2, :]
                                .rearrange("p a b -> p (a b)"),
                                ident)
                        nc.vector.tensor_copy(
                            out=OT[:, :, ic * 512 + isub * 128:
                                   ic * 512 + (isub + 1) * 128],
                            in_=tp2)
                    for tl in range(4):
                        d_one(ic * 4 + tl)

                prev = None  # (ic, o_nat) awaiting transpose + projection
                for ic in range(4):
                    o_nat = c_sb.tile([128, 4, HL, 64], BF16, tag="onat",
                                      name="o_nat", bufs=2)
                    for pr in range(2):
                        # interleaved accumulation groups sharing a PSUM bank
                        # must not use start=True (it corrupts the other
                        # groups' partials): zero the bank once, accumulate.
                        po = [c_ps.tile([128, 4, 128], F32, tag=f"o{j}",
                                        name=f"po{j}", bufs=1)
                              for j in range(2)]
                        nc.vector.memset(po[0], 0.0)
                        nc.vector.memset(po[1], 0.0)
                        for jt in range(NT):
                            ps_s = c_ps.tile([128, 1024], F32, tag="s",
                                             name="ps_s", bufs=2)
                            for hp in range(2):
                                po64 = hp * 64
                                nc.tensor.matmul(
                                    ps_s[:, hp * 512:(hp + 1) * 512],
                                    lhsT=kT[po64:po64 + 64, pr,
                                            jt * 128:(jt + 1) * 128],
                                    rhs=qT[po64:po64 + 64, pr,
                                           ic * 512:(ic + 1) * 512],
                                    start=True, stop=True)
                            ex = c_sb.tile([128, 1024], BF16, tag="ex",
                                           name="ex", bufs=6)
                            nc.scalar.activation(out=ex, in_=ps_s,
                                                 func=AF.Exp, scale=SCALE)
                            for hp in range(2):
                                for isub in range(4):
                                    nc.tensor.matmul(
                                        po[hp][:, isub, 0:66],
                                        lhsT=ex[:, hp * 512 + isub * 128:
                                                hp * 512 + (isub + 1) * 128],
                                        rhs=vext[:, jt, pr * 2 + hp, :],
                                        start=False, stop=(jt == NT - 1),
                                        skip_group_check=True)
                            # drip-feed the previous chunk's epilogue between
                            # early j-iterations so PE work hides under ACT
                            if prev is not None and pr == 0 and jt == 1:
                                finish_ic(*prev)
                                prev = None
                        # normalize: per-partition scalar 1/r
                        for hp in range(2):
                            for isub in range(4):
                                rcp = c_sb.tile([128, 1], F32, tag="rcp",
                                                name="rcp", bufs=8)
                                nc.vector.reciprocal(
                                    out=rcp, in_=po[hp][:, isub, 64:65])
                                nc.vector.tensor_scalar(
                                    out=o_nat[:, isub, pr * 2 + hp, :],
                                    in0=po[hp][:, isub, 0:64],
                                    scalar1=rcp, scalar2=None, op0=ALU.mult)
                    prev = (ic, o_nat)
                finish_ic(*prev)

    nc.compile()
    return nc


def kernel(x, gamma, beta, w_qkv, w_out, b_out):
    """Full inputs in, full output out.  Shards batch x head-groups over 8
    cores, runs the SPMD Bass kernel, and sums the partial projections."""
    import ml_dtypes
    bf16 = ml_dtypes.bfloat16

    if "nc" not in _NC_CACHE:
        _NC_CACHE["nc"] = _build()
    nc = _NC_CACHE["nc"]

    x = np.asarray(x, dtype=np.float32)
    gamma = np.asarray(gamma, dtype=np.float32)
    w_qkv = np.asarray(w_qkv, dtype=np.float32)
    w_out = np.asarray(w_out, dtype=np.float32)
    b_out = np.asarray(b_out, dtype=np.float32)

    wg = w_qkv * gamma[:, None]  # fold LN gamma into the QKV projection
    in_maps = []
    for core in range(8):
        b, g = core // 4, core % 4
        cs = slice(g * CI, (g + 1) * CI)
        in_maps.append({
            "x": np.ascontiguousarray(x[b]),
            "wq": np.ascontiguousarray(wg[:, 0 * 1024:1 * 1024][:, cs]).astype(bf16),
            "wk": np.ascontiguousarray(wg[:, 1 * 1024:2 * 1024][:, cs]).astype(bf16),
            "wv": np.ascontiguousarray(wg[:, 2 * 1024:3 * 1024][:, cs]).astype(bf16),
            "wo": np.ascontiguousarray(w_out[cs, :]).astype(bf16),
        })

    res = bass_utils.run_bass_kernel_spmd(nc, in_maps, core_ids=list(range(8)))
    parts = [r["out"] for r in res.results]
    full = np.stack([
        parts[0] + parts[1] + parts[2] + parts[3],
        parts[4] + parts[5] + parts[6] + parts[7],
    ]).astype(np.float32)
    return full + b_out


# revision 6
# speedup vs baseline: 1.2769x; 1.0653x over previous
"""Fused LayerNorm + multi-head attention + out-projection for Trainium2.

Problem: x[2,2048,1024] -> LN -> QKV (16 heads, dh=64) -> softmax attention
-> out proj.  Sharded over 8 NeuronCores as batch(2) x head-groups(4)
(Megatron tensor parallel): each core handles one batch entry and 4 heads,
computing a partial out-projection; the host sums the 4 partials per batch.

Per-core dataflow (T=2048 tokens, D=1024, 4 local heads, dh=64), bf16
matmul inputs everywhere (fp32 psum accumulation):
  A) per 128-token tile: LN via bn_stats/bn_aggr (fp32), xn cast to bf16,
     PE-transpose to xnT [d, tok].
  B) per 512-token chunk: v natural [tok, c] via lhsT=xnT; qT/kT [c, tok]
     via lhsT=w chunks.  All bf16.
  C) per (i-chunk 512, head-pair): for each j-tile: S^T[j,i] paired matmul
     (two heads on disjoint 64-row PE groups); exp((1/8)S) on ACT -> bf16;
     O[i, d|r] += ex_slice.T @ (V|1) for each 128-token i-sub -- M=128,
     N=66 bf16 matmuls accumulate over j in PSUM; softmax denominator r
     lands in column 64.  Normalize with per-partition scalar 1/r (DVE),
     PE-transpose O back to OT [c, i].
  D) per token tile: out = OT.T @ wo chunks, DMA out.
The exp stream on the scalar engine (~1 elem/cycle/lane) is the wall-clock
floor; all PE/DVE work is sized to hide under it.
gamma is folded into w_qkv on the host; beta/b_out are zeros by spec
(b_out still added on the host).
"""
import numpy as np

import concourse.bacc as bacc
import concourse.mybir as mybir
import concourse.tile as tile
from concourse import bass_utils
from concourse.masks import make_identity

F32 = mybir.dt.float32
BF16 = mybir.dt.bfloat16
AF = mybir.ActivationFunctionType
ALU = mybir.AluOpType

T = 2048          # tokens per core (one batch entry)
D = 1024          # model dim
HL = 4            # local heads per core
DH = 64           # head dim
CI = HL * DH      # local inner dim = 256
NT = T // 128     # 16 token tiles
NK = D // 128     # 8 dim chunks
LN_EPS = 1e-5
SCALE = DH ** -0.5

_NC_CACHE = {}


def _build():
    nc = bacc.Bacc("TRN2", target_bir_lowering=False, debug=False)

    x = nc.dram_tensor("x", [T, D], F32, kind="ExternalInput")
    wq = nc.dram_tensor("wq", [D, CI], BF16, kind="ExternalInput")
    wk = nc.dram_tensor("wk", [D, CI], BF16, kind="ExternalInput")
    wv = nc.dram_tensor("wv", [D, CI], BF16, kind="ExternalInput")
    wo = nc.dram_tensor("wo", [CI, D], BF16, kind="ExternalInput")
    out = nc.dram_tensor("out", [T, D], F32, kind="ExternalOutput")

    x_t = x.rearrange("(t p) d -> t p d", p=128)          # [16, 128, 1024]
    out_t = out.rearrange("(t p) d -> t p d", p=128)
    wq_t = wq.rearrange("(c p) n -> p c n", p=128)        # [128, 8, 256]
    wk_t = wk.rearrange("(c p) n -> p c n", p=128)
    wv_t = wv.rearrange("(c p) n -> p c n", p=128)
    wo_t = wo.rearrange("(c p) n -> p c n", p=128)        # [128, 2, 1024]

    with tile.TileContext(nc) as tc:
        with tc.tile_pool(name="persist", bufs=1) as persist:
            # constants
            ident_f = persist.tile([128, 128], F32, name="ident_f")
            make_identity(nc, ident_f)
            ident = persist.tile([128, 128], BF16, name="ident")
            nc.vector.tensor_copy(out=ident, in_=ident_f)
            eps = persist.tile([128, 1], F32, name="eps")
            nc.vector.memset(eps, LN_EPS)

            # persistent activations / weights (all bf16)
            qT = persist.tile([128, 2, T], BF16, name="qT")     # 8KB/p
            kT = persist.tile([128, 2, T], BF16, name="kT")     # 8KB/p
            vext = persist.tile([128, NT, HL, 66], BF16, name="vext")
            OT = persist.tile([128, 2, T], BF16, name="OT")     # 8KB/p
            wq_s = persist.tile([128, NK, CI], BF16, name="wq_s")
            wk_s = persist.tile([128, NK, CI], BF16, name="wk_s")
            wv_s = persist.tile([128, NK, CI], BF16, name="wv_s")
            wo_s = persist.tile([128, 2, D], BF16, name="wo_s")

            # ones column (64) of vext; column 65 is zero padding
            nc.vector.memset(vext[:, :, :, 64:65], 1.0)
            nc.vector.memset(vext[:, :, :, 65:66], 0.0)

            # ------------- Phase A/B: LN + transpose + QKV -------------
            with (
                tc.tile_pool(name="ab_sb", bufs=3) as ab_sb,
                tc.tile_pool(name="ab_sm", bufs=8) as ab_sm,
                tc.tile_pool(name="ab_ps", bufs=1, space="PSUM") as ab_ps,
            ):
                # prefetch first token tiles, then weights
                xts = {}
                for tt in range(4):
                    xt0 = ab_sb.tile([128, D], F32, tag="xt", name="xt", bufs=4)
                    (nc.sync if tt % 2 == 0 else nc.scalar).dma_start(
                        xt0, x_t[tt])
                    xts[tt] = xt0
                nc.sync.dma_start(wq_s, wq_t)
                nc.sync.dma_start(wk_s, wk_t)
                nc.sync.dma_start(wv_s, wv_t)
                nc.sync.dma_start(wo_s, wo_t)

                for ic in range(4):
                    xnT = ab_sb.tile([128, NK, 512], BF16, tag="xnT",
                                     name="xnT", bufs=2)
                    for tl in range(4):
                        tt = ic * 4 + tl
                        if tt in xts:
                            xt = xts.pop(tt)
                        else:
                            xt = ab_sb.tile([128, D], F32, tag="xt", name="xt",
                                            bufs=4)
                            (nc.sync if tt % 2 == 0 else nc.scalar).dma_start(
                                xt, x_t[tt])
                        stats = ab_sm.tile([128, 2, 6], F32, tag="stats",
                                           name="stats")
                        xr = xt.rearrange("p (c f) -> p c f", f=512)
                        for c in range(2):
                            nc.vector.bn_stats(out=stats[:, c, :], in_=xr[:, c, :])
                        mv = ab_sm.tile([128, 2], F32, tag="mv", name="mv")
                        nc.vector.bn_aggr(out=mv, in_=stats)
                        rstd = ab_sm.tile([128, 1], F32, tag="rstd", name="rstd")
                        nc.scalar.activation(out=rstd, in_=mv[:, 1:2], func=AF.Sqrt,
                                             bias=eps, scale=1.0)
                        nc.vector.reciprocal(out=rstd, in_=rstd)
                        xn = ab_sb.tile([128, D], BF16, tag="xn", name="xn")
                        nc.vector.tensor_scalar(out=xn, in0=xt, scalar1=mv[:, 0:1],
                                                scalar2=rstd, op0=ALU.subtract,
                                                op1=ALU.mult)
                        for kc4 in range(2):
                            pt = ab_ps.tile([128, 4, 128], BF16, tag="tp",
                                            name="pt", bufs=2)
                            for q in range(4):
                                kc = kc4 * 4 + q
                                nc.tensor.transpose(
                                    pt[:, q, :],
                                    xn[:, kc * 128:(kc + 1) * 128],
                                    ident)
                            nc.vector.tensor_copy(
                                out=xnT[:, kc4 * 4:kc4 * 4 + 4,
                                        tl * 128:(tl + 1) * 128], in_=pt)

                    # v natural for these 4 token tiles
                    for tl in range(4):
                        tt = ic * 4 + tl
                        pv = ab_ps.tile([128, CI], F32, tag="pv", name="pv",
                                        bufs=2)
                        for kc in range(NK):
                            nc.tensor.matmul(
                                pv,
                                lhsT=xnT[:, kc, tl * 128:(tl + 1) * 128],
                                rhs=wv_s[:, kc, :],
                                start=(kc == 0), stop=(kc == NK - 1))
                        nc.scalar.copy(
                            out=vext[:, tt, :, 0:64],
                            in_=pv.rearrange("p (h d) -> p h d", h=HL))

                    # qT/kT columns for this token chunk
                    pq = ab_ps.tile([128, 4, 512], F32, tag="pq", name="pq",
                                    bufs=1)
                    for kc in range(NK):
                        for pc in range(4):
                            w_src = wq_s if pc < 2 else wk_s
                            off = (pc % 2) * 128
                            nc.tensor.matmul(
                                pq[:, pc, :],
                                lhsT=w_src[:, kc, off:off + 128],
                                rhs=xnT[:, kc, :],
                                start=(kc == 0), stop=(kc == NK - 1))
                    isl = slice(ic * 512, (ic + 1) * 512)
                    nc.scalar.copy(out=qT[:, 0, isl], in_=pq[:, 0, :])
                    nc.scalar.copy(out=qT[:, 1, isl], in_=pq[:, 1, :])
                    nc.scalar.copy(out=kT[:, 0, isl], in_=pq[:, 2, :])
                    nc.scalar.copy(out=kT[:, 1, isl], in_=pq[:, 3, :])

            # ---------------- Phase C: attention ----------------
            with (
                tc.tile_pool(name="c_sb", bufs=1) as c_sb,
                tc.tile_pool(name="c_ps", bufs=1, space="PSUM") as c_ps,
            ):
                def d_one(tt):
                    # one out-projection token tile
                    pd = c_ps.tile([128, 1024], F32, tag="s", name="pd",
                                   bufs=2)
                    for ncn in range(2):
                        for ck in range(2):
                            nc.tensor.matmul(
                                pd[:, ncn * 512:(ncn + 1) * 512],
                                lhsT=OT[:, ck, tt * 128:(tt + 1) * 128],
                                rhs=wo_s[:, ck, ncn * 512:(ncn + 1) * 512],
                                start=(ck == 0), stop=(ck == 1))
                    ot_sb = c_sb.tile([128, 1024], F32, tag="ot", name="ot_sb",
                                      bufs=3)
                    nc.vector.tensor_copy(out=ot_sb, in_=pd)
                    nc.sync.dma_start(out_t[tt], ot_sb)

                def t_one(ic, o_nat, isub):
                    # transpose normalized O [i, c] -> OT [c, i] for one i-sub
                    tp2 = c_ps.tile([128, 2, 128], BF16, tag="tp2",
                                    name="tp2", bufs=2)
                    for ck in range(2):
                        nc.tensor.transpose(
                            tp2[:, ck, :],
                            o_nat[:, isub, 2 * ck:2 * ck + 2, :]
                            .rearrange("p a b -> p (a b)"),
                            ident)
                    nc.vector.tensor_copy(
                        out=OT[:, :, ic * 512 + isub * 128:
                               ic * 512 + (isub + 1) * 128],
                        in_=tp2)

                prev = None  # (ic, o_nat) awaiting transpose + projection
                for ic in range(4):
                    o_nat = c_sb.tile([128, 4, HL, 64], BF16, tag="onat",
                                      name="o_nat", bufs=2)
                    for pr in range(2):
                        # epilogue of the previous i-chunk, split into small
                        # PE chunks dripped between j-iterations (slot id ->
                        # work) so they hide in PE idle under the ACT stream
                        drip = {}
                        if prev is not None:
                            pic, pon = prev
                            for i in range(4):
                                drip[2 + 2 * i] = (t_one, (pic, pon, i))
                                drip[3 + 2 * i] = (d_one, (pic * 4 + i,))
                            prev = None

                        # interleaved accumulation groups sharing a PSUM bank
                        # must not use start=True (it corrupts the other
                        # groups' partials): zero the bank once, accumulate.
                        po = [c_ps.tile([128, 4, 128], F32, tag=f"o{j}",
                                        name=f"po{j}", bufs=1)
                              for j in range(2)]
                        nc.vector.memset(po[0], 0.0)
                        nc.vector.memset(po[1], 0.0)

                        def o_one(jt, exv):
                            for hp in range(2):
                                for isub in range(4):
                                    nc.tensor.matmul(
                                        po[hp][:, isub, 0:66],
                                        lhsT=exv[:, hp * 512 + isub * 128:
                                                 hp * 512 + (isub + 1) * 128],
                                        rhs=vext[:, jt, pr * 2 + hp, :],
                                        start=False, stop=(jt == NT - 1),
                                        skip_group_check=True)

                        # software-pipelined: issue S(jt)+exp(jt) ahead of
                        # O(jt-1) so the PE never queues behind the ACT
                        # dependency and the exp stream stays saturated
                        exs = {}
                        for jt in range(NT + 1):
                            if jt < NT:
                                ps_s = c_ps.tile([128, 1024], F32, tag="s",
                                                 name="ps_s", bufs=2)
                                for hp in range(2):
                                    po64 = hp * 64
                                    nc.tensor.matmul(
                                        ps_s[:, hp * 512:(hp + 1) * 512],
                                        lhsT=kT[po64:po64 + 64, pr,
                                                jt * 128:(jt + 1) * 128],
                                        rhs=qT[po64:po64 + 64, pr,
                                               ic * 512:(ic + 1) * 512],
                                        start=True, stop=True)
                                ex = c_sb.tile([128, 1024], BF16, tag="ex",
                                               name="ex", bufs=6)
                                nc.scalar.activation(out=ex, in_=ps_s,
                                                     func=AF.Exp, scale=SCALE)
                                exs[jt] = ex
                            if jt > 0:
                                o_one(jt - 1, exs.pop(jt - 1))
                            if jt in drip:
                                fn, args = drip.pop(jt)
                                fn(*args)
                        # normalize: per-partition scalar 1/r
                        for hp in range(2):
                            for isub in range(4):
                                rcp = c_sb.tile([128, 1], F32, tag="rcp",
                                                name="rcp", bufs=8)
                                nc.vector.reciprocal(
                                    out=rcp, in_=po[hp][:, isub, 64:65])
                                nc.vector.tensor_scalar(
                                    out=o_nat[:, isub, pr * 2 + hp, :],
                                    in0=po[hp][:, isub, 0:64],
                                    scalar1=rcp, scalar2=None, op0=ALU.mult)
                    prev = (ic, o_nat)
                pic, pon = prev
                for i in range(4):
                    t_one(pic, pon, i)
                for i in range(4):
                    d_one(pic * 4 + i)

    nc.compile()
    return nc


def kernel(x, gamma, beta, w_qkv, w_out, b_out):
    """Full inputs in, full output out.  Shards batch x head-groups over 8
    cores, runs the SPMD Bass kernel, and sums the partial projections."""
    import ml_dtypes
    bf16 = ml_dtypes.bfloat16

    if "nc" not in _NC_CACHE:
        _NC_CACHE["nc"] = _build()
    nc = _NC_CACHE["nc"]

    x = np.asarray(x, dtype=np.float32)
    gamma = np.asarray(gamma, dtype=np.float32)
    w_qkv = np.asarray(w_qkv, dtype=np.float32)
    w_out = np.asarray(w_out, dtype=np.float32)
    b_out = np.asarray(b_out, dtype=np.float32)

    wg = w_qkv * gamma[:, None]  # fold LN gamma into the QKV projection
    in_maps = []
    for core in range(8):
        b, g = core // 4, core % 4
        cs = slice(g * CI, (g + 1) * CI)
        in_maps.append({
            "x": np.ascontiguousarray(x[b]),
            "wq": np.ascontiguousarray(wg[:, 0 * 1024:1 * 1024][:, cs]).astype(bf16),
            "wk": np.ascontiguousarray(wg[:, 1 * 1024:2 * 1024][:, cs]).astype(bf16),
            "wv": np.ascontiguousarray(wg[:, 2 * 1024:3 * 1024][:, cs]).astype(bf16),
            "wo": np.ascontiguousarray(w_out[cs, :]).astype(bf16),
        })

    res = bass_utils.run_bass_kernel_spmd(nc, in_maps, core_ids=list(range(8)))
    parts = [r["out"] for r in res.results]
    full = np.stack([
        parts[0] + parts[1] + parts[2] + parts[3],
        parts[4] + parts[5] + parts[6] + parts[7],
    ]).astype(np.float32)
    return full + b_out


# revision 9
# speedup vs baseline: 1.3576x; 1.0632x over previous
"""Fused LayerNorm + multi-head attention + out-projection for Trainium2.

Problem: x[2,2048,1024] -> LN -> QKV (16 heads, dh=64) -> softmax attention
-> out proj.  Sharded over 8 NeuronCores as batch(2) x head-groups(4)
(Megatron tensor parallel): each core handles one batch entry and 4 heads,
computing a partial out-projection; the host sums the 4 partials per batch.

Per-core dataflow (T=2048 tokens, D=1024, 4 local heads, dh=64), bf16
matmul inputs everywhere (fp32 psum accumulation):
  A) per 128-token tile: LN via bn_stats/bn_aggr (fp32), xn cast to bf16,
     PE-transpose to persistent xnT [d, tok].
  B) v natural [tok, c] for all 4 heads and qT/kT [c, tok] for head pair 0.
  C) per (head-pair, i-chunk 512): software-pipelined over j-tiles:
     S^T[j,i] paired matmul (two heads on disjoint 64-row PE groups);
     exp((1/8)S) on ACT -> bf16; O[i, d|r] += ex_slice.T @ (V|1) as M=128,
     N=66 bf16 matmuls accumulating in PSUM (denominator r in column 64).
     The scalar-engine exp stream is the wall-clock floor; leftover PE
     work (pair-1 q/k projections during pr=0, O-transposes + output
     projections during pr=1) is dripped one instruction per j-step into
     the PE idle gaps so the tensor engine stays busy enough to hold the
     HAM clock at 2.4 GHz and never stalls the exp stream.
  Normalization is a per-partition scalar multiply (1/r) on DVE in [i, d]
  layout, then PE-transpose back to OT [c, i] for the out-projection.
gamma is folded into w_qkv on the host; beta/b_out are zeros by spec
(b_out still added on the host).
"""
import numpy as np

import concourse.bacc as bacc
import concourse.mybir as mybir
import concourse.tile as tile
from concourse import bass_utils
from concourse.masks import make_identity

F32 = mybir.dt.float32
BF16 = mybir.dt.bfloat16
AF = mybir.ActivationFunctionType
ALU = mybir.AluOpType

T = 2048          # tokens per core (one batch entry)
D = 1024          # model dim
HL = 4            # local heads per core
DH = 64           # head dim
CI = HL * DH      # local inner dim = 256
NT = T // 128     # 16 token tiles
NK = D // 128     # 8 dim chunks
LN_EPS = 1e-5
SCALE = DH ** -0.5

_NC_CACHE = {}


def _build():
    nc = bacc.Bacc("TRN2", target_bir_lowering=False, debug=False)

    x = nc.dram_tensor("x", [T, D], F32, kind="ExternalInput")
    wq = nc.dram_tensor("wq", [D, CI], BF16, kind="ExternalInput")
    wk = nc.dram_tensor("wk", [D, CI], BF16, kind="ExternalInput")
    wv = nc.dram_tensor("wv", [D, CI], BF16, kind="ExternalInput")
    wo = nc.dram_tensor("wo", [CI, D], BF16, kind="ExternalInput")
    out = nc.dram_tensor("out", [T, D], F32, kind="ExternalOutput")

    x_t = x.rearrange("(t p) d -> t p d", p=128)          # [16, 128, 1024]
    out_t = out.rearrange("(t p) d -> t p d", p=128)
    wq_t = wq.rearrange("(c p) n -> p c n", p=128)        # [128, 8, 256]
    wk_t = wk.rearrange("(c p) n -> p c n", p=128)
    wv_t = wv.rearrange("(c p) n -> p c n", p=128)
    wo_t = wo.rearrange("(c p) n -> p c n", p=128)        # [128, 2, 1024]

    with tile.TileContext(nc) as tc:
        with tc.tile_pool(name="persist", bufs=1) as persist:
            # constants
            ident_f = persist.tile([128, 128], F32, name="ident_f")
            make_identity(nc, ident_f)
            ident = persist.tile([128, 128], BF16, name="ident")
            nc.vector.tensor_copy(out=ident, in_=ident_f)
            eps = persist.tile([128, 1], F32, name="eps")
            nc.vector.memset(eps, LN_EPS)

            # persistent activations / weights (all bf16)
            xnT = persist.tile([128, NK, T], BF16, name="xnT")  # 32KB/p
            qT = persist.tile([128, 2, T], BF16, name="qT")
            kT = persist.tile([128, 2, T], BF16, name="kT")
            vext = persist.tile([128, NT, HL, 66], BF16, name="vext")
            OT = persist.tile([128, 2, T], BF16, name="OT")
            o_nat = persist.tile([128, 4, 4, HL, 64], BF16, name="o_nat")
            wq_s = persist.tile([128, NK, CI], BF16, name="wq_s")
            wk_s = persist.tile([128, NK, CI], BF16, name="wk_s")
            wv_s = persist.tile([128, NK, CI], BF16, name="wv_s")
            wo_s = persist.tile([128, 2, D], BF16, name="wo_s")

            # ones column (64) of vext; column 65 is zero padding
            nc.vector.memset(vext[:, :, :, 64:65], 1.0)
            nc.vector.memset(vext[:, :, :, 65:66], 0.0)

            # ---- Phase A: LN + transpose; B0: v (all heads) + q/k pair 0
            with (
                tc.tile_pool(name="ab_sb", bufs=3) as ab_sb,
                tc.tile_pool(name="ab_sm", bufs=8) as ab_sm,
                tc.tile_pool(name="ab_ps", bufs=1, space="PSUM") as ab_ps,
            ):
                xts = {}
                for tt in range(4):
                    xt0 = ab_sb.tile([128, D], F32, tag="xt", name="xt", bufs=4)
                    (nc.sync if tt % 2 == 0 else nc.scalar).dma_start(
                        xt0, x_t[tt])
                    xts[tt] = xt0
                nc.sync.dma_start(wq_s, wq_t)
                nc.sync.dma_start(wk_s, wk_t)
                nc.sync.dma_start(wv_s, wv_t)
                nc.sync.dma_start(wo_s, wo_t)

                for ic in range(4):
                    for tl in range(4):
                        tt = ic * 4 + tl
                        if tt in xts:
                            xt = xts.pop(tt)
                        else:
                            xt = ab_sb.tile([128, D], F32, tag="xt", name="xt",
                                            bufs=4)
                            (nc.sync if tt % 2 == 0 else nc.scalar).dma_start(
                                xt, x_t[tt])
                        stats = ab_sm.tile([128, 2, 6], F32, tag="stats",
                                           name="stats")
                        xr = xt.rearrange("p (c f) -> p c f", f=512)
                        for c in range(2):
                            nc.vector.bn_stats(out=stats[:, c, :], in_=xr[:, c, :])
                        mv = ab_sm.tile([128, 2], F32, tag="mv", name="mv")
                        nc.vector.bn_aggr(out=mv, in_=stats)
                        rstd = ab_sm.tile([128, 1], F32, tag="rstd", name="rstd")
                        nc.scalar.activation(out=rstd, in_=mv[:, 1:2], func=AF.Sqrt,
                                             bias=eps, scale=1.0)
                        nc.vector.reciprocal(out=rstd, in_=rstd)
                        xn = ab_sb.tile([128, D], BF16, tag="xn", name="xn")
                        nc.vector.tensor_scalar(out=xn, in0=xt, scalar1=mv[:, 0:1],
                                                scalar2=rstd, op0=ALU.subtract,
                                                op1=ALU.mult)
                        for kc4 in range(2):
                            pt = ab_ps.tile([128, 4, 128], BF16, tag="tp",
                                            name="pt", bufs=2)
                            for q in range(4):
                                kc = kc4 * 4 + q
                                nc.tensor.transpose(
                                    pt[:, q, :],
                                    xn[:, kc * 128:(kc + 1) * 128],
                                    ident)
                            nc.vector.tensor_copy(
                                out=xnT[:, kc4 * 4:kc4 * 4 + 4,
                                        tt * 128:(tt + 1) * 128], in_=pt)

                    # v natural (all 4 heads) for these 4 token tiles
                    for tl in range(4):
                        tt = ic * 4 + tl
                        pv = ab_ps.tile([128, CI], F32, tag="pv", name="pv",
                                        bufs=2)
                        for kc in range(NK):
                            nc.tensor.matmul(
                                pv,
                                lhsT=xnT[:, kc, tt * 128:(tt + 1) * 128],
                                rhs=wv_s[:, kc, :],
                                start=(kc == 0), stop=(kc == NK - 1))
                        nc.scalar.copy(
                            out=vext[:, tt, :, 0:64],
                            in_=pv.rearrange("p (h d) -> p h d", h=HL))

                    # qT/kT pair 0 for this token chunk
                    isl = slice(ic * 512, (ic + 1) * 512)
                    pq = ab_ps.tile([128, 2, 512], F32, tag="pq", name="pq",
                                    bufs=2)
                    for kc in range(NK):
                        for pc in range(2):
                            w_src = wq_s if pc == 0 else wk_s
                            nc.tensor.matmul(
                                pq[:, pc, :],
                                lhsT=w_src[:, kc, 0:128],
                                rhs=xnT[:, kc, isl],
                                start=(kc == 0), stop=(kc == NK - 1))
                    nc.scalar.copy(out=qT[:, 0, isl], in_=pq[:, 0, :])
                    nc.scalar.copy(out=kT[:, 0, isl], in_=pq[:, 1, :])

            # ---------------- Phase C: attention ----------------
            with (
                tc.tile_pool(name="c_sb", bufs=1) as c_sb,
                tc.tile_pool(name="c_ps", bufs=1, space="PSUM") as c_ps,
            ):
                def d_one(tt):
                    # one out-projection token tile
                    pd = c_ps.tile([128, 1024], F32, tag="s", name="pd",
                                   bufs=2)
                    for ncn in range(2):
                        for ck in range(2):
                            nc.tensor.matmul(
                                pd[:, ncn * 512:(ncn + 1) * 512],
                                lhsT=OT[:, ck, tt * 128:(tt + 1) * 128],
                                rhs=wo_s[:, ck, ncn * 512:(ncn + 1) * 512],
                                start=(ck == 0), stop=(ck == 1))
                    ot_sb = c_sb.tile([128, 1024], F32, tag="ot", name="ot_sb",
                                      bufs=3)
                    nc.vector.tensor_copy(out=ot_sb, in_=pd)
                    nc.sync.dma_start(out_t[tt], ot_sb)

                def t_one(ic, isub):
                    # transpose normalized O [i, c] -> OT [c, i] for one i-sub
                    tp2 = c_ps.tile([128, 2, 128], BF16, tag="aux",
                                    name="tp2", bufs=1)
                    for ck in range(2):
                        nc.tensor.transpose(
                            tp2[:, ck, :],
                            o_nat[:, ic, isub, 2 * ck:2 * ck + 2, :]
                            .rearrange("p a b -> p (a b)"),
                            ident)
                    nc.vector.tensor_copy(
                        out=OT[:, :, ic * 512 + isub * 128:
                               ic * 512 + (isub + 1) * 128],
                        in_=tp2)

                for pr in range(2):
                    for ic in range(4):
                        isl = slice(ic * 512, (ic + 1) * 512)
                        # drip-feed schedule: one small PE task per j-step,
                        # keeping the tensor engine busy under the exp stream
                        drip = {}
                        if pr == 0:
                            # pair-1 q/k projections for this token chunk
                            pq1 = c_ps.tile([128, 2, 512], F32, tag="aux",
                                            name="pq1", bufs=1)

                            def qk1(kc, pc, _pq1=pq1, _isl=isl):
                                w_src = wq_s if pc == 0 else wk_s
                                nc.tensor.matmul(
                                    _pq1[:, pc, :],
                                    lhsT=w_src[:, kc, 128:256],
                                    rhs=xnT[:, kc, _isl],
                                    start=(kc == 0), stop=(kc == NK - 1))

                            def qk1_copy(pc, _pq1=pq1, _isl=isl):
                                dst = qT if pc == 0 else kT
                                nc.vector.tensor_copy(out=dst[:, 1, _isl],
                                                      in_=_pq1[:, pc, :])

                            for kc in range(NK):
                                drip[kc] = (qk1, (kc, 0))
                                drip[NK + kc] = (qk1, (kc, 1))
                        else:
                            # epilogue of chunk ic-1: transposes + projection
                            if ic > 0:
                                for i in range(4):
                                    drip[1 + 2 * i] = (t_one, (ic - 1, i))
                                    drip[2 + 2 * i] = (d_one, ((ic - 1) * 4 + i,))

                        po = [c_ps.tile([128, 4, 128], F32, tag=f"o{j}",
                                        name=f"po{j}", bufs=1)
                              for j in range(2)]
                        nc.vector.memset(po[0], 0.0)
                        nc.vector.memset(po[1], 0.0)

                        def o_one(jt, exv):
                            for hp in range(2):
                                for isub in range(4):
                                    nc.tensor.matmul(
                                        po[hp][:, isub, 0:66],
                                        lhsT=exv[:, hp * 512 + isub * 128:
                                                 hp * 512 + (isub + 1) * 128],
                                        rhs=vext[:, jt, pr * 2 + hp, :],
                                        start=False, stop=(jt == NT - 1),
                                        skip_group_check=True)

                        # software-pipelined: issue S(jt)+exp(jt) ahead of
                        # O(jt-1) so the PE never queues behind the ACT
                        # dependency and the exp stream stays saturated
                        exs = {}
                        for jt in range(NT + 1):
                            if jt < NT:
                                ps_s = c_ps.tile([128, 1024], F32, tag="s",
                                                 name="ps_s", bufs=2)
                                for hp in range(2):
                                    po64 = hp * 64
                                    nc.tensor.matmul(
                                        ps_s[:, hp * 512:(hp + 1) * 512],
                                        lhsT=kT[po64:po64 + 64, pr,
                                                jt * 128:(jt + 1) * 128],
                                        rhs=qT[po64:po64 + 64, pr, isl],
                                        start=True, stop=True)
                                ex = c_sb.tile([128, 1024], BF16, tag="ex",
                                               name="ex", bufs=6)
                                nc.scalar.activation(out=ex, in_=ps_s,
                                                     func=AF.Exp, scale=SCALE)
                                exs[jt] = ex
                            if jt > 0:
                                o_one(jt - 1, exs.pop(jt - 1))
                            if jt in drip:
                                fn, args = drip.pop(jt)
                                fn(*args)
                        # pair-1 q/k copies (DVE) after their accumulations
                        if pr == 0:
                            qk1_copy(0)
                            qk1_copy(1)
                        # normalize: per-partition scalar 1/r
                        for hp in range(2):
                            for isub in range(4):
                                rcp = c_sb.tile([128, 1], F32, tag="rcp",
                                                name="rcp", bufs=8)
                                nc.vector.reciprocal(
                                    out=rcp, in_=po[hp][:, isub, 64:65])
                                nc.vector.tensor_scalar(
                                    out=o_nat[:, ic, isub, pr * 2 + hp, :],
                                    in0=po[hp][:, isub, 0:64],
                                    scalar1=rcp, scalar2=None, op0=ALU.mult)
                # tail: epilogue of the last chunk
                for i in range(4):
                    t_one(3, i)
                for i in range(4):
                    d_one(12 + i)

    nc.compile()
    return nc


def kernel(x, gamma, beta, w_qkv, w_out, b_out):
    """Full inputs in, full output out.  Shards batch x head-groups over 8
    cores, runs the SPMD Bass kernel, and sums the partial projections."""
    import ml_dtypes
    bf16 = ml_dtypes.bfloat16

    if "nc" not in _NC_CACHE:
        _NC_CACHE["nc"] = _build()
    nc = _NC_CACHE["nc"]

    x = np.asarray(x, dtype=np.float32)
    gamma = np.asarray(gamma, dtype=np.float32)
    w_qkv = np.asarray(w_qkv, dtype=np.float32)
    w_out = np.asarray(w_out, dtype=np.float32)
    b_out = np.asarray(b_out, dtype=np.float32)

    wg = w_qkv * gamma[:, None]  # fold LN gamma into the QKV projection
    in_maps = []
    for core in range(8):
        b, g = core // 4, core % 4
        cs = slice(g * CI, (g + 1) * CI)
        in_maps.append({
            "x": np.ascontiguousarray(x[b]),
            "wq": np.ascontiguousarray(wg[:, 0 * 1024:1 * 1024][:, cs]).astype(bf16),
            "wk": np.ascontiguousarray(wg[:, 1 * 1024:2 * 1024][:, cs]).astype(bf16),
            "wv": np.ascontiguousarray(wg[:, 2 * 1024:3 * 1024][:, cs]).astype(bf16),
            "wo": np.ascontiguousarray(w_out[cs, :]).astype(bf16),
        })

    res = bass_utils.run_bass_kernel_spmd(nc, in_maps, core_ids=list(range(8)))
    parts = [r["out"] for r in res.results]
    full = np.stack([
        parts[0] + parts[1] + parts[2] + parts[3],
        parts[4] + parts[5] + parts[6] + parts[7],
    ]).astype(np.float32)
    return full + b_out


# revision 15
# speedup vs baseline: 1.4750x; 1.0865x over previous
"""Fused LayerNorm + multi-head attention + out-projection for Trainium2.

Problem: x[2,2048,1024] -> LN -> QKV (16 heads, dh=64) -> softmax attention
-> out proj.  Sharded over 8 NeuronCores as batch(2) x head-groups(4)
(Megatron tensor parallel): each core handles one batch entry and 4 heads,
computing a partial out-projection; the host sums the 4 partials per batch.

Per-core dataflow (T=2048 tokens, D=1024, 4 local heads, dh=64), bf16
matmul inputs everywhere (fp32 psum accumulation):
  A) per 128-token tile: LN via bn_stats/bn_aggr (fp32), xn cast to bf16,
     PE-transpose to persistent xnT [d, tok].
  B) v natural [tok, c] for all 4 heads and qT/kT [c, tok] for head pair 0.
  C) per (head-pair, i-chunk 512): software-pipelined over j-tiles:
     S^T[j,i] paired matmul (two heads on disjoint 64-row PE groups);
     exp((1/8)S) on ACT -> bf16; O[i, d|r] += ex_slice.T @ (V|1) as M=128,
     N=66 bf16 matmuls accumulating in PSUM (denominator r in column 64).
     The scalar-engine exp stream is the wall-clock floor; leftover PE
     work (pair-1 q/k projections during pr=0, O-transposes + output
     projections during pr=1) is dripped one instruction per j-step into
     the PE idle gaps so the tensor engine stays busy enough to hold the
     HAM clock at 2.4 GHz and never stalls the exp stream.
  Normalization is a per-partition scalar multiply (1/r) on DVE in [i, d]
  layout, then PE-transpose back to OT [c, i] for the out-projection.
gamma is folded into w_qkv on the host; beta/b_out are zeros by spec
(b_out still added on the host).
"""
import numpy as np

import concourse.bacc as bacc
import concourse.mybir as mybir
import concourse.tile as tile
from concourse import bass_utils
from concourse.bass import broadcast_tensor_aps
from concourse.masks import make_identity

F32 = mybir.dt.float32
BF16 = mybir.dt.bfloat16
AF = mybir.ActivationFunctionType
ALU = mybir.AluOpType

T = 2048          # tokens per core (one batch entry)
D = 1024          # model dim
HL = 4            # local heads per core
DH = 64           # head dim
CI = HL * DH      # local inner dim = 256
NT = T // 128     # 16 token tiles
NK = D // 128     # 8 dim chunks
LN_EPS = 1e-5
SCALE = DH ** -0.5

_NC_CACHE = {}


def _build():
    nc = bacc.Bacc("TRN2", target_bir_lowering=False, debug=False)

    x = nc.dram_tensor("x", [T, D], F32, kind="ExternalInput")
    wq = nc.dram_tensor("wq", [D, CI], BF16, kind="ExternalInput")
    wk = nc.dram_tensor("wk", [D, CI], BF16, kind="ExternalInput")
    wv = nc.dram_tensor("wv", [D, CI], BF16, kind="ExternalInput")
    wo = nc.dram_tensor("wo", [CI, D], BF16, kind="ExternalInput")
    out = nc.dram_tensor("out", [T, D], F32, kind="ExternalOutput")

    x_t = x.rearrange("(t p) d -> t p d", p=128)          # [16, 128, 1024]
    out_t = out.rearrange("(t p) d -> t p d", p=128)
    wq_t = wq.rearrange("(c p) n -> p c n", p=128)        # [128, 8, 256]
    wk_t = wk.rearrange("(c p) n -> p c n", p=128)
    wv_t = wv.rearrange("(c p) n -> p c n", p=128)
    wo_t = wo.rearrange("(c p) n -> p c n", p=128)        # [128, 2, 1024]

    with tile.TileContext(nc) as tc:
        with tc.tile_pool(name="persist", bufs=1) as persist:
            # constants
            ident_f = persist.tile([128, 128], F32, name="ident_f")
            make_identity(nc, ident_f)
            ident = persist.tile([128, 128], BF16, name="ident")
            nc.vector.tensor_copy(out=ident, in_=ident_f)
            eps = persist.tile([128, 1], F32, name="eps")
            nc.vector.memset(eps, LN_EPS)

            # persistent activations / weights (all bf16)
            xnT = persist.tile([128, NK, T], BF16, name="xnT")  # 32KB/p
            qT = persist.tile([128, 2, T], BF16, name="qT")
            kT = persist.tile([128, 2, T], BF16, name="kT")
            vext = persist.tile([128, NT, HL, 66], BF16, name="vext")
            OT = persist.tile([128, 2, T], BF16, name="OT")
            o_nat = persist.tile([128, 4, 4, HL, 64], BF16, name="o_nat")
            wq_s = persist.tile([128, NK, CI], BF16, name="wq_s")
            wk_s = persist.tile([128, NK, CI], BF16, name="wk_s")
            wv_s = persist.tile([128, NK, CI], BF16, name="wv_s")
            wo_s = persist.tile([128, 2, D], BF16, name="wo_s")

            # ones column (64) of vext; column 65 is zero padding
            nc.vector.memset(vext[:, :, :, 64:65], 1.0)
            nc.vector.memset(vext[:, :, :, 65:66], 0.0)

            # ---- Phase A: LN + transpose; B0: v (all heads) + q/k pair 0
            with (
                tc.tile_pool(name="ab_sb", bufs=3) as ab_sb,
                tc.tile_pool(name="ab_sm", bufs=8) as ab_sm,
                tc.tile_pool(name="ab_ps", bufs=1, space="PSUM") as ab_ps,
            ):
                xts = {}
                for tt in range(4):
                    xt0 = ab_sb.tile([128, D], F32, tag="xt", name="xt", bufs=4)
                    (nc.sync if tt % 2 == 0 else nc.scalar).dma_start(
                        xt0, x_t[tt])
                    xts[tt] = xt0
                nc.sync.dma_start(wq_s, wq_t)
                nc.sync.dma_start(wk_s, wk_t)
                nc.sync.dma_start(wv_s, wv_t)
                nc.sync.dma_start(wo_s, wo_t)

                for ic in range(4):
                    for tl in range(4):
                        tt = ic * 4 + tl
                        if tt in xts:
                            xt = xts.pop(tt)
                        else:
                            xt = ab_sb.tile([128, D], F32, tag="xt", name="xt",
                                            bufs=4)
                            (nc.sync if tt % 2 == 0 else nc.scalar).dma_start(
                                xt, x_t[tt])
                        stats = ab_sm.tile([128, 2, 6], F32, tag="stats",
                                           name="stats")
                        xr = xt.rearrange("p (c f) -> p c f", f=512)
                        for c in range(2):
                            nc.vector.bn_stats(out=stats[:, c, :], in_=xr[:, c, :])
                        mv = ab_sm.tile([128, 2], F32, tag="mv", name="mv")
                        nc.vector.bn_aggr(out=mv, in_=stats)
                        rstd = ab_sm.tile([128, 1], F32, tag="rstd", name="rstd")
                        nc.scalar.activation(out=rstd, in_=mv[:, 1:2], func=AF.Sqrt,
                                             bias=eps, scale=1.0)
                        nc.vector.reciprocal(out=rstd, in_=rstd)
                        nmr = ab_sm.tile([128, 1], F32, tag="nmr", name="nmr")
                        nc.vector.tensor_scalar(out=nmr, in0=mv[:, 0:1],
                                                scalar1=rstd, scalar2=-1.0,
                                                op0=ALU.mult, op1=ALU.mult)
                        # xn = x*rstd - mu*rstd on ACT (Identity is in the
                        # sqrt table set, so no extra table load)
                        xn = ab_sb.tile([128, D], BF16, tag="xn", name="xn")
                        nc.scalar.activation(out=xn, in_=xt, func=AF.Identity,
                                             bias=nmr, scale=rstd)
                        for kc4 in range(2):
                            pt = ab_ps.tile([128, 4, 128], BF16, tag="tp",
                                            name="pt", bufs=2)
                            for q in range(4):
                                kc = kc4 * 4 + q
                                nc.tensor.transpose(
                                    pt[:, q, :],
                                    xn[:, kc * 128:(kc + 1) * 128],
                                    ident)
                            nc.vector.tensor_copy(
                                out=xnT[:, kc4 * 4:kc4 * 4 + 4,
                                        tt * 128:(tt + 1) * 128], in_=pt)

                    # v natural (all 4 heads) for these 4 token tiles
                    for tl in range(4):
                        tt = ic * 4 + tl
                        pv = ab_ps.tile([128, CI], F32, tag="pv", name="pv",
                                        bufs=2)
                        for kc in range(NK):
                            nc.tensor.matmul(
                                pv,
                                lhsT=xnT[:, kc, tt * 128:(tt + 1) * 128],
                                rhs=wv_s[:, kc, :],
                                start=(kc == 0), stop=(kc == NK - 1))
                        nc.scalar.copy(
                            out=vext[:, tt, :, 0:64],
                            in_=pv.rearrange("p (h d) -> p h d", h=HL))

                    # qT/kT pair 0 for this token chunk
                    isl = slice(ic * 512, (ic + 1) * 512)
                    pq = ab_ps.tile([128, 2, 512], F32, tag="pq", name="pq",
                                    bufs=2)
                    for kc in range(NK):
                        for pc in range(2):
                            w_src = wq_s if pc == 0 else wk_s
                            nc.tensor.matmul(
                                pq[:, pc, :],
                                lhsT=w_src[:, kc, 0:128],
                                rhs=xnT[:, kc, isl],
                                start=(kc == 0), stop=(kc == NK - 1))
                    nc.scalar.copy(out=qT[:, 0, isl], in_=pq[:, 0, :])
                    nc.scalar.copy(out=kT[:, 0, isl], in_=pq[:, 1, :])

            # ---------------- Phase C: attention ----------------
            with (
                tc.tile_pool(name="c_sb", bufs=1) as c_sb,
                tc.tile_pool(name="c_ps", bufs=1, space="PSUM") as c_ps,
            ):
                def d_one(tt):
                    # one out-projection token tile
                    pd = c_ps.tile([128, 1024], F32, tag="s", name="pd",
                                   bufs=2)
                    for ncn in range(2):
                        for ck in range(2):
                            nc.tensor.matmul(
                                pd[:, ncn * 512:(ncn + 1) * 512],
                                lhsT=OT[:, ck, tt * 128:(tt + 1) * 128],
                                rhs=wo_s[:, ck, ncn * 512:(ncn + 1) * 512],
                                start=(ck == 0), stop=(ck == 1))
                    ot_sb = c_sb.tile([128, 1024], F32, tag="ot", name="ot_sb",
                                      bufs=3)
                    nc.vector.tensor_copy(out=ot_sb, in_=pd)
                    nc.sync.dma_start(out_t[tt], ot_sb)

                def t_one(ic, isub):
                    # transpose normalized O [i, c] -> OT [c, i] for one i-sub
                    tp2 = c_ps.tile([128, 2, 128], BF16, tag="aux",
                                    name="tp2", bufs=2)
                    for ck in range(2):
                        nc.tensor.transpose(
                            tp2[:, ck, :],
                            o_nat[:, ic, isub, 2 * ck:2 * ck + 2, :]
                            .rearrange("p a b -> p (a b)"),
                            ident)
                    nc.vector.tensor_copy(
                        out=OT[:, :, ic * 512 + isub * 128:
                               ic * 512 + (isub + 1) * 128],
                        in_=tp2)

                for pr in range(2):
                    for ic in range(4):
                        isl = slice(ic * 512, (ic + 1) * 512)
                        # drip-feed schedule: one small PE task per j-step,
                        # keeping the tensor engine busy under the exp stream
                        drip = {}
                        if pr == 0:
                            # pair-1 q/k projections for this token chunk;
                            # q and k each use a 1-bank aux psum allocation
                            pq1 = {}

                            def qk1(kc, pc, _isl=isl):
                                if kc == 0:
                                    pq1[pc] = c_ps.tile([128, 512], F32,
                                                        tag="aux", name="pq1",
                                                        bufs=2)
                                w_src = wq_s if pc == 0 else wk_s
                                nc.tensor.matmul(
                                    pq1[pc],
                                    lhsT=w_src[:, kc, 128:256],
                                    rhs=xnT[:, kc, _isl],
                                    start=(kc == 0), stop=(kc == NK - 1))

                            def qk1_copy(pc, _isl=isl):
                                dst = qT if pc == 0 else kT
                                nc.vector.tensor_copy(out=dst[:, 1, _isl],
                                                      in_=pq1.pop(pc))

                            for kc in range(NK):
                                drip[kc] = (qk1, (kc, 0))
                                drip[NK + kc] = (qk1, (kc, 1))
                            drip[NK + NK] = (qk1_copy, (0,))
                        else:
                            # epilogue of chunk ic-1: transposes + projection
                            if ic > 0:
                                for i in range(4):
                                    drip[1 + 2 * i] = (t_one, (ic - 1, i))
                                    drip[2 + 2 * i] = (d_one, ((ic - 1) * 4 + i,))

                        po = [c_ps.tile([128, 4, 128], F32, tag=f"o{j}",
                                        name=f"po{j}", bufs=1)
                              for j in range(2)]
                        nc.vector.memset(po[0], 0.0)
                        nc.vector.memset(po[1], 0.0)

                        def o_one(jt, exv):
                            for hp in range(2):
                                for isub in range(4):
                                    nc.tensor.matmul(
                                        po[hp][:, isub, 0:66],
                                        lhsT=exv[:, hp * 512 + isub * 128:
                                                 hp * 512 + (isub + 1) * 128],
                                        rhs=vext[:, jt, pr * 2 + hp, :],
                                        start=False, stop=(jt == NT - 1),
                                        skip_group_check=True)

                        # software-pipelined: issue S(jt)+exp(jt) ahead of
                        # O(jt-1) so the PE never queues behind the ACT
                        # dependency and the exp stream stays saturated
                        exs = {}
                        for jt in range(NT + 1):
                            if jt < NT:
                                ps_s = c_ps.tile([128, 1024], F32, tag="s",
                                                 name="ps_s", bufs=2)
                                for hp in range(2):
                                    po64 = hp * 64
                                    nc.tensor.matmul(
                                        ps_s[:, hp * 512:(hp + 1) * 512],
                                        lhsT=kT[po64:po64 + 64, pr,
                                                jt * 128:(jt + 1) * 128],
                                        rhs=qT[po64:po64 + 64, pr, isl],
                                        start=True, stop=True)
                                ex = c_sb.tile([128, 1024], BF16, tag="ex",
                                               name="ex", bufs=8)
                                nc.scalar.activation(out=ex, in_=ps_s,
                                                     func=AF.Exp, scale=SCALE)
                                exs[jt] = ex
                            if jt > 0:
                                o_one(jt - 1, exs.pop(jt - 1))
                            if jt in drip:
                                fn, args = drip.pop(jt)
                                fn(*args)
                        # pair-1 k copy (DVE) after its accumulation
                        if pr == 0:
                            qk1_copy(1)
                        # normalize: batched per-(partition, i-sub) scalar
                        # 1/r via a 0-stride broadcast tensor_tensor
                        for hp in range(2):
                            rcp4 = c_sb.tile([128, 4, 1], F32, tag="rcp",
                                             name="rcp4", bufs=4)
                            nc.vector.reciprocal(out=rcp4,
                                                 in_=po[hp][:, :, 64:65])
                            dst = o_nat[:, ic, :, pr * 2 + hp, :]
                            src = po[hp][:, :, 0:64]
                            rb, _ = broadcast_tensor_aps(rcp4[:, :, :], src)
                            nc.vector.tensor_tensor(out=dst, in0=src, in1=rb,
                                                    op=ALU.mult)
                # tail: epilogue of the last chunk
                for i in range(4):
                    t_one(3, i)
                for i in range(4):
                    d_one(12 + i)

    nc.compile()
    return nc


def kernel(x, gamma, beta, w_qkv, w_out, b_out):
    """Full inputs in, full output out.  Shards batch x head-groups over 8
    cores, runs the SPMD Bass kernel, and sums the partial projections."""
    import ml_dtypes
    bf16 = ml_dtypes.bfloat16

    if "nc" not in _NC_CACHE:
        _NC_CACHE["nc"] = _build()
    nc = _NC_CACHE["nc"]

    x = np.asarray(x, dtype=np.float32)
    gamma = np.asarray(gamma, dtype=np.float32)
    w_qkv = np.asarray(w_qkv, dtype=np.float32)
    w_out = np.asarray(w_out, dtype=np.float32)
    b_out = np.asarray(b_out, dtype=np.float32)

    wg = w_qkv * gamma[:, None]  # fold LN gamma into the QKV projection
    in_maps = []
    for core in range(8):
        b, g = core // 4, core % 4
        cs = slice(g * CI, (g + 1) * CI)
        in_maps.append({
            "x": np.ascontiguousarray(x[b]),
            "wq": np.ascontiguousarray(wg[:, 0 * 1024:1 * 1024][:, cs]).astype(bf16),
            "wk": np.ascontiguousarray(wg[:, 1 * 1024:2 * 1024][:, cs]).astype(bf16),
            "wv": np.ascontiguousarray(wg[:, 2 * 1024:3 * 1024][:, cs]).astype(bf16),
            "wo": np.ascontiguousarray(w_out[cs, :]).astype(bf16),
        })

    res = bass_utils.run_bass_kernel_spmd(nc, in_maps, core_ids=list(range(8)))
    parts = [r["out"] for r in res.results]
    full = np.stack([
        parts[0] + parts[1] + parts[2] + parts[3],
        parts[4] + parts[5] + parts[6] + parts[7],
    ]).astype(np.float32)
    return full + b_out
